# revision 56
# baseline (speedup 1.0000x reference)
"""Block-diagonal GRU cell for Trainium2, data-parallel over 8 NeuronCores.

Math (per batch row b, block j of 8, block size 256):
    wx  = x @ W_ir.T + b_ir_lin + b_ir          # [B, 6144], gates r|z|n global-chunked
    wh  = hb_j @ W_h[j].T + b_hr_j              # per block, local r|z|n chunks of 256
    r   = sigmoid(wxr + whr)
    z   = sigmoid(wxz + whz)
    n   = tanh(wxn + r * whn)
    h'  = (1-z)*hb + z*n

Device strategy (per core, batch-sharded 1024 rows):
  - Mixed fp8/fp16 matmuls, chosen from a measured per-path error budget
    (L2-relative output error if only that path is e4m3-quantized):
        wxr 1.3e-3 | wxz 9.8e-3 | wxn 1.55e-2 | whr 6.5e-4 | whz 4.9e-3 | whn 4.1e-3
    wxn dominates, so it stays fp16; the other five paths run e4m3 with
    MatmulPerfMode.DoubleRow (two K=128 chunks per pass, 2x PE rate).
    Measured end-to-end rel err ~1.25e-2 vs the 2e-2 gate.
  - Scaling: e4m3 needs the operands lifted out of denormal range, so
    activations carry x16 and weights x256 (PSUM = 4096 * logical). The
    fp16 wxn operands are scaled identically (exact powers of two), so
    both PSUM banks are uniformly 4096-scaled and the descale folds into
    the two activation-scale factors (1/4096 for r|z, 2/4096 for the
    tanh-as-sigmoid trick).
  - Blocks are processed in PAIRS (j, j+1) per m-tile: the PSUM tiles are
    two banks each (A2 = r|z sums for both blocks, B2 = [wxn|whn] for
    both), so every epilogue op is 512-1024 wide instead of 256-512.
    DVE/ACT ops pay ~200ns fixed latency each; doubling the width halves
    that overhead per element. Stores become one contiguous 512-col DMA
    per pair.
  - Epilogue (per pair): rz=sig(A2); t3=r*whn; t4=wxn+t3; tn=sig(2*t4);
    e=2*tn-hb1 (=n-hb, one STT against the GpSimd-precomputed hb1=h+1);
    t5=z*e; out=t5+h. Intermediates are fp16 so the non-PSUM DVE ops hit
    the 2x_1port mode; h and out are fp16 end-to-end (blend error ~2e-4,
    negligible vs the fp8 matmul error; halves that DMA traffic).
  - hb1 = h+1 runs on the otherwise-idle GpSimd right behind each h-tile
    DMA, far off the epilogue's critical path (Pool's V3 ISA only allows
    plain TENSOR_TENSOR, and its sequencer is too slow for the serial
    chain itself).
  - Every DRAM tensor is host-relaid so each DMA reads AND writes >=512B
    contiguous per partition (m-major x columns, block-major weight
    columns, (pair,m)-major h^T tiles): small strided runs were measured
    to cap the sync HWDGE ring at ~180GB/s, starving the pair pipeline.
  - All loads ride the SP HWDGE ring; stores ride the ACT ring (disjoint
    FIFOs, so prefetch-blocked loads never delay epilogue slot releases).
    Except the final pair: by then the sync ring is idle while ACT still
    has work queued ahead in its FIFO.
"""

import sys

if "/opt/trn_rl_repo" not in sys.path:
    sys.path.insert(0, "/opt/trn_rl_repo")

import numpy as np
import ml_dtypes

B, IN, H, NB = 8192, 1024, 2048, 8
BS = H // NB  # 256
NCORES = 8
BC = B // NCORES  # 1024 rows per core
P = 128
K1 = IN // P  # 8 x-projection contraction chunks
K2 = BS // P  # 2 h-projection contraction chunks per block
MT = BC // P  # 8 m-tiles per core
NP = NB // 2  # 4 block-pairs
CN = 4  # wxn k-chunks kept in fp16; the rest run e4m3 DoubleRow
SX = 16.0  # activation pre-scale (fp8 and fp16 operands)
SW = 256.0  # weight pre-scale
SC = SX * SW  # PSUM carries 4096 * logical value

_BUILD_CACHE = {}


def build_nc(bc=BC, has_bias=False):
    """Build the Bass program for one core (SPMD: same program on all 8)."""
    key = (bc, has_bias)
    if key in _BUILD_CACHE:
        return _BUILD_CACHE[key]

    from contextlib import ExitStack

    import concourse.bacc as bacc
    import concourse.mybir as mybir
    import concourse.tile as tile

    f8 = mybir.dt.float8e4
    f16 = mybir.dt.float16
    f32 = mybir.dt.float32
    SIG = mybir.ActivationFunctionType.Sigmoid
    MULT = mybir.AluOpType.mult
    SUB = mybir.AluOpType.subtract
    DR = mybir.MatmulPerfMode.DoubleRow

    mt = bc // P

    # Bacc (not plain Bass): its compile() runs move_matmul_waits_to_ldweights
    # + generate_event_semaphores, which split multi-sem waits down to the
    # 1-wait-per-instruction TRN2 ISA budget.
    nc = bacc.Bacc(target_bir_lowering=False)

    # all dram tensors are pre-tiled on the host: leading index selects a
    # [128, contiguous] panel
    xt8 = nc.dram_tensor("xt8", [mt * P, K1 * P], f8, kind="ExternalInput").ap()
    xt16 = nc.dram_tensor("xt16", [mt * P, K1 * P], f16, kind="ExternalInput").ap()
    ht8 = nc.dram_tensor(
        "ht8", [NP * mt * P, 2 * K2 * P], f8, kind="ExternalInput"
    ).ap()
    h16 = nc.dram_tensor("h16", [bc, H], f16, kind="ExternalInput").ap()
    wrz = nc.dram_tensor("wrz", [NB * P, K1 * 2 * BS], f8, kind="ExternalInput").ap()
    wn = nc.dram_tensor("wn", [NB * P, CN * BS], f16, kind="ExternalInput").ap()
    wn8 = nc.dram_tensor("wn8", [NB * P, (K1 - CN) * BS], f8, kind="ExternalInput").ap()
    whrz = nc.dram_tensor(
        "whrz", [NB * P, K2 * 2 * BS], f8, kind="ExternalInput"
    ).ap()
    whn = nc.dram_tensor("whn", [NB * P, K2 * BS], f8, kind="ExternalInput").ap()
    if has_bias:
        brz_d = nc.dram_tensor("brz", [1, NB * 2 * BS], f32, kind="ExternalInput").ap()
        bxn_d = nc.dram_tensor("bxn", [1, NB * BS], f32, kind="ExternalInput").ap()
        bhn_d = nc.dram_tensor("bhn", [1, NB * BS], f32, kind="ExternalInput").ap()
    out = nc.dram_tensor("out", [bc, H], f16, kind="ExternalOutput").ap()

    def prow(t, i):
        return t[i * P : (i + 1) * P, :]

    # panel-major views: [128, panel-index, contiguous bytes]
    xt8_v = xt8.rearrange("(m p) c -> p m c", p=P)  # [128, mt, K1*P]
    xt16_v = xt16.rearrange("(m p) c -> p m c", p=P)
    ht8_v = ht8.rearrange("(a p) c -> p a c", p=P)  # [128, NP*mt, 2*K2*P]
    h16_v = h16.rearrange("(m p) c -> p m c", p=P)  # [128, mt, H]

    with tile.TileContext(nc) as tc, ExitStack() as ctx:
        wpool = ctx.enter_context(tc.tile_pool(name="wres", bufs=1))
        spool = ctx.enter_context(tc.tile_pool(name="stream", bufs=mt + mt // 2))
        psA = ctx.enter_context(tc.tile_pool(name="psA", bufs=2, space="PSUM"))
        psB = ctx.enter_context(tc.tile_pool(name="psB", bufs=2, space="PSUM"))
        epool = ctx.enter_context(tc.tile_pool(name="epi", bufs=6))

        # ---- resident tiles (m-major / block-major so every DMA panel is
        # contiguous on both sides) ----
        xt8_sb = wpool.tile([P, mt, K1, P], f8, tag="xt8_sb")
        xt16_sb = wpool.tile([P, mt, K1, P], f16, tag="xt16_sb")
        wrz_sb = wpool.tile([P, NB, K1, 2 * BS], f8, tag="wrz_sb")
        wn_sb = wpool.tile([P, NB, CN, BS], f16, tag="wn_sb")
        wn8_sb = wpool.tile([P, NB, (K1 - CN) // 2, 2, BS], f8, tag="wn8_sb")
        whrz_sb = wpool.tile([P, NB, K2, 2 * BS], f8, tag="whrz_sb")
        whn_sb = wpool.tile([P, NB, K2, BS], f8, tag="whn_sb")

        def load_wh_col(j):
            # h-projection weights: block j's h-side passes are the group
            # openers, so these small columns load first
            nc.sync.dma_start(whrz_sb[:, j, :, :], prow(whrz, j))
            nc.sync.dma_start(whn_sb[:, j, :, :], prow(whn, j))

        def load_wx_col(j):
            nc.sync.dma_start(wrz_sb[:, j, :, :], prow(wrz, j))
            nc.sync.dma_start(wn_sb[:, j, :, :], prow(wn, j))
            nc.sync.dma_start(wn8_sb[:, j, :, :, :], prow(wn8, j))

        def load_mp_streams(m, jp):
            # one ht + one h DMA per (m, block-pair) -- mid-size DMAs keep
            # all 16 DMA engines busy (one DMA maps to ONE engine at
            # ~22GB/s, so whole-tensor transfers serialize); hb1 = h + 1 is
            # precomputed by the otherwise-idle GpSimd right behind the h
            # DMA, well off the epilogue's critical path
            msl = slice(m * P, (m + 1) * P)
            psl = slice(2 * jp * BS, (2 * jp + 2) * BS)
            ht_mp = spool.tile([P, 2 * K2, P], f8, tag="ht_mp")
            nc.sync.dma_start(ht_mp[:, :, :], prow(ht8, jp * mt + m))
            h_mp = spool.tile([P, 2, BS], f16, tag="h_mp")
            nc.sync.dma_start(h_mp[:, :, :], h16[msl, psl])
            hb1_mp = spool.tile([P, 2, BS], f16, tag="hb1_mp")
            nc.gpsimd.tensor_add(hb1_mp[:], h_mp[:], ones2_sb[:])
            return ht_mp, h_mp, hb1_mp

        # prewarm the ACT sigmoid table (~2.7us ACT_TABLE_LOAD) at t~0 so
        # the first real epilogue doesn't pay it inline right when the PE's
        # PSUM bank rotation depends on that sigmoid releasing bank A
        ws = wpool.tile([P, 1], f32, tag="ws")
        nc.vector.memset(ws[:], 0.0)
        nc.scalar.activation(ws[:], ws[:], SIG)
        ones2_sb = wpool.tile([P, 2, BS], f16, tag="ones2_sb")
        nc.gpsimd.memset(ones2_sb[:], 1.0)

        # head, ordered by need-time: pair 0's h-weights + m0 streams, m0's
        # x columns + block 0/1 x-weights, then the remaining per-m streams
        streams = {}
        load_wh_col(0)
        load_wh_col(1)
        streams[(0, 0)] = load_mp_streams(0, 0)
        nc.sync.dma_start(xt8_sb[:, 0, :, :], prow(xt8, 0))
        nc.sync.dma_start(wrz_sb[:, 0, :, :], prow(wrz, 0))
        nc.sync.dma_start(xt16_sb[:, 0, :, :], prow(xt16, 0))
        nc.sync.dma_start(wn_sb[:, 0, :, :], prow(wn, 0))
        nc.sync.dma_start(wn8_sb[:, 0, :, :, :], prow(wn8, 0))
        load_wx_col(1)
        for m in range(1, mt):
            # xt8 + streams feed the early matmul groups; xt16 (wxn path)
            # is needed last within each (j,m), so it loads after them
            nc.sync.dma_start(xt8_sb[:, m, :, :], prow(xt8, m))
            streams[(m, 0)] = load_mp_streams(m, 0)
            nc.sync.dma_start(xt16_sb[:, m, :, :], prow(xt16, m))
        if has_bias:
            ones_sb = wpool.tile([1, P], f32, tag="ones_sb")
            nc.vector.memset(ones_sb[:], 1.0)
            brz_sb = wpool.tile([1, NB * 2 * BS], f32, tag="brz_sb")
            bxn_sb = wpool.tile([1, NB * BS], f32, tag="bxn_sb")
            bhn_sb = wpool.tile([1, NB * BS], f32, tag="bhn_sb")
            nc.sync.dma_start(brz_sb[:], brz_d[:])
            nc.sync.dma_start(bxn_sb[:], bxn_d[:])
            nc.sync.dma_start(bhn_sb[:], bhn_d[:])

        for jp in range(NP):
            for m in range(mt):
                msl = slice(m * P, (m + 1) * P)
                ht_mp, h_mp, hb1_mp = streams.pop((m, jp))
                A2 = psA.tile([P, 2, 2 * BS], f32, tag="A")
                B2 = psB.tile([P, 2, 2 * BS], f32, tag="B")
                # A2 (r|z) completes first -- all h-passes and rz passes up
                # front, the wxn chain last -- so sigma/t3 overlap the wxn
                # phase and the post-matmul epilogue chain is shorter
                for i in range(2):
                    j = 2 * jp + i
                    # h-projection DoubleRow passes open both banks
                    # (start=True marks the bank pending-zero; exactly one
                    # start per bank half)
                    nc.tensor.matmul(
                        A2[:, i, :], lhsT=ht_mp[:, 2 * i : 2 * i + 2, :],
                        rhs=whrz_sb[:, j, :, :],
                        start=True, stop=False, perf_mode=DR,
                    )
                    nc.tensor.matmul(
                        B2[:, i, BS : 2 * BS],
                        lhsT=ht_mp[:, 2 * i : 2 * i + 2, :],
                        rhs=whn_sb[:, j, :, :],
                        start=True, stop=False, perf_mode=DR,
                    )
                for i in range(2):
                    j = 2 * jp + i
                    # x-projection r|z: 4 DoubleRow passes (K=256 each)
                    for p in range(K1 // 2):
                        nc.tensor.matmul(
                            A2[:, i, :],
                            lhsT=xt8_sb[:, m, 2 * p : 2 * p + 2, :],
                            rhs=wrz_sb[:, j, 2 * p : 2 * p + 2, :],
                            start=False,
                            stop=(p == K1 // 2 - 1) and not has_bias,
                            perf_mode=DR,
                        )
                for i in range(2):
                    j = 2 * jp + i
                    # x-projection n: wxn dominates the error budget, so
                    # CN chunks keep fp16's 10 mantissa bits; the rest run
                    # e4m3 DoubleRow (measured rel err 1.61e-2 at CN=4 vs
                    # 1.18e-2 all-fp16, both under the 2e-2 gate)
                    for k in range(CN):
                        nc.tensor.matmul(
                            B2[:, i, 0:BS], lhsT=xt16_sb[:, m, k, :],
                            rhs=wn_sb[:, j, k, :],
                            start=False, stop=False,
                        )
                    for p in range((K1 - CN) // 2):
                        nc.tensor.matmul(
                            B2[:, i, 0:BS],
                            lhsT=xt8_sb[:, m, CN + 2 * p : CN + 2 * p + 2, :],
                            rhs=wn8_sb[:, j, p, :, :],
                            start=False,
                            stop=(p == (K1 - CN) // 2 - 1) and not has_bias,
                            perf_mode=DR,
                        )
                    if has_bias:
                        jrz = slice(j * 2 * BS, (j + 1) * 2 * BS)
                        jn = slice(j * BS, (j + 1) * BS)
                        # rank-1 bias add: ones[K=1,128].T @ bias[K=1,N]
                        # (biases host-pre-scaled by 4096 to match PSUM units)
                        nc.tensor.matmul(
                            A2[:, i, :], lhsT=ones_sb[:, :], rhs=brz_sb[:, jrz],
                            start=False, stop=True,
                        )
                        nc.tensor.matmul(
                            B2[:, i, 0:BS], lhsT=ones_sb[:, :], rhs=bxn_sb[:, jn],
                            start=False, stop=False,
                        )
                        nc.tensor.matmul(
                            B2[:, i, BS : 2 * BS], lhsT=ones_sb[:, :],
                            rhs=bhn_sb[:, jn],
                            start=False, stop=True,
                        )

                # pair-wide epilogue on ACT + DVE only; fp16 off-PSUM so the
                # back-half DVE ops run the 2x_1port mode. The final few
                # m-tiles emit per-block (half-width) epilogues instead:
                # the drain after the last matmul is bounded by the serial
                # sigma->t3->..->store chain, and halving the op width
                # halves that latency.
                def epilogue(i0, ni, tg):
                    isl = slice(i0, i0 + ni)
                    rz2 = epool.tile([P, ni, 2 * BS], f16, tag="rz" + tg)
                    nc.scalar.activation(
                        rz2[:], A2[:, isl, :], SIG, scale=1.0 / SC
                    )
                    t3 = epool.tile([P, ni, BS], f16, tag="t3" + tg)
                    nc.vector.tensor_mul(
                        t3[:], rz2[:, :, 0:BS], B2[:, isl, BS : 2 * BS]
                    )
                    t4 = epool.tile([P, ni, BS], f16, tag="t4" + tg)
                    nc.vector.tensor_add(t4[:], B2[:, isl, 0:BS], t3[:])
                    tn = epool.tile([P, ni, BS], f16, tag="tn" + tg)
                    nc.scalar.activation(tn[:], t4[:], SIG, scale=2.0 / SC)
                    # n - hb = 2*sigmoid(2y) - (hb + 1), one STT
                    e = epool.tile([P, ni, BS], f16, tag="e" + tg)
                    nc.vector.scalar_tensor_tensor(
                        e[:], tn[:], 2.0, hb1_mp[:, isl, :], op0=MULT, op1=SUB
                    )
                    t5 = epool.tile([P, ni, BS], f16, tag="t5" + tg)
                    nc.vector.tensor_mul(t5[:], rz2[:, :, BS : 2 * BS], e[:])
                    oj = epool.tile([P, ni, BS], f16, tag="t3" + tg)
                    nc.vector.tensor_add(oj[:], t5[:], h_mp[:, isl, :])
                    # stores ride the ACT ring except the final pair (sync
                    # is idle by then, ACT still has a backlog in its FIFO)
                    osl = slice((2 * jp + i0) * BS, (2 * jp + i0 + ni) * BS)
                    if jp == NP - 1:
                        nc.sync.dma_start(out[msl, osl], oj[:])
                    else:
                        nc.scalar.dma_start(out[msl, osl], oj[:])

                epilogue(0, 2, "")
                # this m's pair tiles just released: prefetch its next-pair
                # streams now so the slot-wait never blocks the DMA FIFO
                if jp + 1 < NP:
                    streams[(m, jp + 1)] = load_mp_streams(m, jp + 1)
                # next pair's weight columns, spread over the early m-tiles
                if jp + 1 < NP and m < 2:
                    load_wh_col(2 * (jp + 1) + m)
                    load_wx_col(2 * (jp + 1) + m)

    nc.compile()
    _BUILD_CACHE[key] = nc
    return nc


def _q8(a, scale):
    return np.clip(np.float32(a) * np.float32(scale), -240.0, 240.0).astype(
        ml_dtypes.float8_e4m3
    )


def prep_inputs(x, h, W_ir, b_ir_lin, b_ir, W_h, b_hr, ncores=NCORES):
    """Host-side reshaping/casting -> per-core in_maps + has_bias flag."""
    x = np.asarray(x, dtype=np.float32)
    h = np.asarray(h, dtype=np.float32)
    W_ir = np.asarray(W_ir, dtype=np.float32)
    W_h = np.asarray(W_h, dtype=np.float32)
    b_ir_lin = np.asarray(b_ir_lin, dtype=np.float32)
    b_ir = np.asarray(b_ir, dtype=np.float32)
    b_hr = np.asarray(b_hr, dtype=np.float32)

    bc = x.shape[0] // ncores
    mt = bc // P

    # weights: gate-and-block reordered, pre-scaled, then re-tiled so each
    # block column is one [128, contiguous] DMA panel
    Wr = W_ir[0:H].reshape(NB, BS, IN)
    Wz = W_ir[H : 2 * H].reshape(NB, BS, IN)
    Wn_ = W_ir[2 * H :].reshape(NB, BS, IN)
    wrz_f = (
        np.concatenate([Wr, Wz], axis=1)  # [NB, 512, IN]
        .transpose(2, 0, 1)
        .reshape(IN, NB * 2 * BS)
    )
    wn_f = Wn_.transpose(2, 0, 1).reshape(IN, NB * BS) * SW
    whrz_f = W_h[:, 0 : 2 * BS, :].transpose(2, 0, 1).reshape(BS, NB * 2 * BS)
    whn_f = W_h[:, 2 * BS :, :].transpose(2, 0, 1).reshape(BS, NB * BS)

    def wtile(w, kk, cols):  # [kk*P, NB*cols] -> [NB*P, kk*cols] block-major
        return np.ascontiguousarray(
            w.reshape(kk, P, NB, cols).transpose(2, 1, 0, 3).reshape(NB * P, kk * cols)
        )

    wrz = wtile(_q8(wrz_f, SW), K1, 2 * BS)
    # wxn: first CN k-chunks in fp16 (pre-scaled), remainder in e4m3
    wn = wtile(wn_f[0 : CN * P].astype(np.float16), CN, BS)
    wn8 = wtile(_q8(wn_f[CN * P :] / SW, SW), K1 - CN, BS)
    whrz = wtile(_q8(whrz_f, SW), K2, 2 * BS)
    whn = wtile(_q8(whn_f, SW), K2, BS)

    bx = b_ir_lin + b_ir
    bh = b_hr.reshape(NB, 3 * BS)
    brz = np.concatenate(
        [
            bx[0:H].reshape(NB, BS) + bh[:, 0:BS],
            bx[H : 2 * H].reshape(NB, BS) + bh[:, BS : 2 * BS],
        ],
        axis=1,
    ).reshape(1, NB * 2 * BS)
    bxn = bx[2 * H :].reshape(1, NB * BS).copy()
    bhn = bh[:, 2 * BS :].reshape(1, NB * BS).copy()
    has_bias = bool(np.any(brz) or np.any(bxn) or np.any(bhn))

    xT = np.ascontiguousarray(x.T)  # [IN, B]
    hT = np.ascontiguousarray(h.T)  # [H, B]
    xT8 = _q8(xT, SX)
    xT16 = (xT * SX).astype(np.float16)
    hT8 = _q8(hT, SX)

    def xtile(a, csl):  # [K1*P, bc] -> [mt*P, K1*P] m-major panels
        return np.ascontiguousarray(
            a[:, csl]
            .reshape(K1, P, mt, P)
            .transpose(2, 1, 0, 3)
            .reshape(mt * P, K1 * P)
        )

    def htile(a, csl):  # [NP*4*P, bc] -> [NP*mt*P, 4*P] (pair,m)-major
        return np.ascontiguousarray(
            a[:, csl]
            .reshape(NP, 2 * K2, P, mt, P)
            .transpose(0, 3, 2, 1, 4)
            .reshape(NP * mt * P, 2 * K2 * P)
        )

    in_maps = []
    for c in range(ncores):
        csl = slice(c * bc, (c + 1) * bc)
        m = {
            "xt8": xtile(xT8, csl),
            "xt16": xtile(xT16, csl),
            "ht8": htile(hT8, csl),
            "h16": np.ascontiguousarray(h[csl].astype(np.float16)),
            "wrz": wrz,
            "wn": wn,
            "wn8": wn8,
            "whrz": whrz,
            "whn": whn,
        }
        if has_bias:
            # PSUM carries 4096x the logical value, so biases do too
            m["brz"] = (brz * SC).astype(np.float32)
            m["bxn"] = (bxn * SC).astype(np.float32)
            m["bhn"] = (bhn * SC).astype(np.float32)
        in_maps.append(m)
    return in_maps, has_bias, bc


def kernel(x, h, W_ir, b_ir_lin, b_ir, W_h, b_hr):
    from concourse.bass_utils import run_bass_kernel_spmd

    in_maps, has_bias, bc = prep_inputs(x, h, W_ir, b_ir_lin, b_ir, W_h, b_hr)
    nc = build_nc(bc=bc, has_bias=has_bias)
    try:
        res = run_bass_kernel_spmd(nc, in_maps, list(range(NCORES)))
    except Exception:
        # transient NRT device errors have been observed once in ~10 runs;
        # a single retry reuses the compiled NEFF
        res = run_bass_kernel_spmd(nc, in_maps, list(range(NCORES)))
    return np.concatenate(
        [res.results[c]["out"] for c in range(NCORES)], axis=0
    ).astype(np.float32)


# revision 57
# speedup vs baseline: 1.0461x; 1.0461x over previous
"""Block-diagonal GRU cell for Trainium2, data-parallel over 8 NeuronCores.

Math (per batch row b, block j of 8, block size 256):
    wx  = x @ W_ir.T + b_ir_lin + b_ir          # [B, 6144], gates r|z|n global-chunked
    wh  = hb_j @ W_h[j].T + b_hr_j              # per block, local r|z|n chunks of 256
    r   = sigmoid(wxr + whr)
    z   = sigmoid(wxz + whz)
    n   = tanh(wxn + r * whn)
    h'  = (1-z)*hb + z*n

Device strategy (per core, batch-sharded 1024 rows):
  - Mixed fp8/fp16 matmuls, chosen from a measured per-path error budget
    (L2-relative output error if only that path is e4m3-quantized):
        wxr 1.3e-3 | wxz 9.8e-3 | wxn 1.55e-2 | whr 6.5e-4 | whz 4.9e-3 | whn 4.1e-3
    wxn dominates, so it stays fp16; the other five paths run e4m3 with
    MatmulPerfMode.DoubleRow (two K=128 chunks per pass, 2x PE rate).
    Measured end-to-end rel err ~1.25e-2 vs the 2e-2 gate.
  - Scaling: e4m3 needs the operands lifted out of denormal range, so
    activations carry x16 and weights x256 (PSUM = 4096 * logical). The
    fp16 wxn operands are scaled identically (exact powers of two), so
    both PSUM banks are uniformly 4096-scaled and the descale folds into
    the two activation-scale factors (1/4096 for r|z, 2/4096 for the
    tanh-as-sigmoid trick).
  - Blocks are processed in PAIRS (j, j+1) per m-tile: the PSUM tiles are
    two banks each (A2 = r|z sums for both blocks, B2 = [wxn|whn] for
    both), so every epilogue op is 512-1024 wide instead of 256-512.
    DVE/ACT ops pay ~200ns fixed latency each; doubling the width halves
    that overhead per element. Stores become one contiguous 512-col DMA
    per pair.
  - Epilogue (per pair): rz=sig(A2); t3=r*whn; t4=wxn+t3; tn=sig(2*t4);
    e=2*tn-hb1 (=n-hb, one STT against the GpSimd-precomputed hb1=h+1);
    t5=z*e; out=t5+h. Intermediates are fp16 so the non-PSUM DVE ops hit
    the 2x_1port mode; h and out are fp16 end-to-end (blend error ~2e-4,
    negligible vs the fp8 matmul error; halves that DMA traffic).
  - hb1 = h+1 runs on the otherwise-idle GpSimd right behind each h-tile
    DMA, far off the epilogue's critical path (Pool's V3 ISA only allows
    plain TENSOR_TENSOR, and its sequencer is too slow for the serial
    chain itself).
  - Every DRAM tensor is host-relaid so each DMA reads AND writes >=512B
    contiguous per partition (m-major x columns, block-major weight
    columns, (pair,m)-major h^T tiles): small strided runs were measured
    to cap the sync HWDGE ring at ~180GB/s, starving the pair pipeline.
  - All loads ride the SP HWDGE ring; stores ride the ACT ring (disjoint
    FIFOs, so prefetch-blocked loads never delay epilogue slot releases).
    Except the final pair: by then the sync ring is idle while ACT still
    has work queued ahead in its FIFO.
"""

import sys

if "/opt/trn_rl_repo" not in sys.path:
    sys.path.insert(0, "/opt/trn_rl_repo")

import numpy as np
import ml_dtypes

B, IN, H, NB = 8192, 1024, 2048, 8
BS = H // NB  # 256
NCORES = 8
BC = B // NCORES  # 1024 rows per core
P = 128
K1 = IN // P  # 8 x-projection contraction chunks
K2 = BS // P  # 2 h-projection contraction chunks per block
MT = BC // P  # 8 m-tiles per core
NP = NB // 2  # 4 block-pairs
CN = 4  # wxn k-chunks kept in fp16; the rest run e4m3 DoubleRow
SX = 16.0  # activation pre-scale (fp8 and fp16 operands)
SW = 256.0  # weight pre-scale
SC = SX * SW  # PSUM carries 4096 * logical value

_BUILD_CACHE = {}


def build_nc(bc=BC, has_bias=False):
    """Build the Bass program for one core (SPMD: same program on all 8)."""
    key = (bc, has_bias)
    if key in _BUILD_CACHE:
        return _BUILD_CACHE[key]

    from contextlib import ExitStack

    import concourse.bacc as bacc
    import concourse.mybir as mybir
    import concourse.tile as tile

    f8 = mybir.dt.float8e4
    f16 = mybir.dt.float16
    f32 = mybir.dt.float32
    SIG = mybir.ActivationFunctionType.Sigmoid
    MULT = mybir.AluOpType.mult
    SUB = mybir.AluOpType.subtract
    DR = mybir.MatmulPerfMode.DoubleRow

    mt = bc // P

    # Bacc (not plain Bass): its compile() runs move_matmul_waits_to_ldweights
    # + generate_event_semaphores, which split multi-sem waits down to the
    # 1-wait-per-instruction TRN2 ISA budget.
    nc = bacc.Bacc(target_bir_lowering=False)

    # all dram tensors are pre-tiled on the host: leading index selects a
    # [128, contiguous] panel
    xt8 = nc.dram_tensor("xt8", [mt * P, K1 * P], f8, kind="ExternalInput").ap()
    xt16 = nc.dram_tensor("xt16", [mt * P, K1 * P], f16, kind="ExternalInput").ap()
    ht8 = nc.dram_tensor(
        "ht8", [NP * mt * P, 2 * K2 * P], f8, kind="ExternalInput"
    ).ap()
    h16 = nc.dram_tensor("h16", [bc, H], f16, kind="ExternalInput").ap()
    wrz = nc.dram_tensor("wrz", [NB * P, K1 * 2 * BS], f8, kind="ExternalInput").ap()
    wn = nc.dram_tensor("wn", [NB * P, CN * BS], f16, kind="ExternalInput").ap()
    wn8 = nc.dram_tensor("wn8", [NB * P, (K1 - CN) * BS], f8, kind="ExternalInput").ap()
    whrz = nc.dram_tensor(
        "whrz", [NB * P, K2 * 2 * BS], f8, kind="ExternalInput"
    ).ap()
    whn = nc.dram_tensor("whn", [NB * P, K2 * BS], f8, kind="ExternalInput").ap()
    if has_bias:
        brz_d = nc.dram_tensor("brz", [1, NB * 2 * BS], f32, kind="ExternalInput").ap()
        bxn_d = nc.dram_tensor("bxn", [1, NB * BS], f32, kind="ExternalInput").ap()
        bhn_d = nc.dram_tensor("bhn", [1, NB * BS], f32, kind="ExternalInput").ap()
    out = nc.dram_tensor("out", [bc, H], f16, kind="ExternalOutput").ap()

    def prow(t, i):
        return t[i * P : (i + 1) * P, :]

    # panel-major views: [128, panel-index, contiguous bytes]
    xt8_v = xt8.rearrange("(m p) c -> p m c", p=P)  # [128, mt, K1*P]
    xt16_v = xt16.rearrange("(m p) c -> p m c", p=P)
    ht8_v = ht8.rearrange("(a p) c -> p a c", p=P)  # [128, NP*mt, 2*K2*P]
    h16_v = h16.rearrange("(m p) c -> p m c", p=P)  # [128, mt, H]

    with tile.TileContext(nc) as tc, ExitStack() as ctx:
        wpool = ctx.enter_context(tc.tile_pool(name="wres", bufs=1))
        spool = ctx.enter_context(tc.tile_pool(name="stream", bufs=mt + mt // 2))
        psA = ctx.enter_context(tc.tile_pool(name="psA", bufs=2, space="PSUM"))
        psB = ctx.enter_context(tc.tile_pool(name="psB", bufs=2, space="PSUM"))
        epool = ctx.enter_context(tc.tile_pool(name="epi", bufs=6))

        # ---- resident tiles (m-major / block-major so every DMA panel is
        # contiguous on both sides) ----
        xt8_sb = wpool.tile([P, mt, K1, P], f8, tag="xt8_sb")
        xt16_sb = wpool.tile([P, mt, K1, P], f16, tag="xt16_sb")
        wrz_sb = wpool.tile([P, NB, K1, 2 * BS], f8, tag="wrz_sb")
        wn_sb = wpool.tile([P, NB, CN, BS], f16, tag="wn_sb")
        wn8_sb = wpool.tile([P, NB, (K1 - CN) // 2, 2, BS], f8, tag="wn8_sb")
        whrz_sb = wpool.tile([P, NB, K2, 2 * BS], f8, tag="whrz_sb")
        whn_sb = wpool.tile([P, NB, K2, BS], f8, tag="whn_sb")

        def load_wh_col(j):
            # h-projection weights: block j's h-side passes are the group
            # openers, so these small columns load first
            nc.sync.dma_start(whrz_sb[:, j, :, :], prow(whrz, j))
            nc.sync.dma_start(whn_sb[:, j, :, :], prow(whn, j))

        def load_wx_col(j):
            nc.sync.dma_start(wrz_sb[:, j, :, :], prow(wrz, j))
            nc.sync.dma_start(wn_sb[:, j, :, :], prow(wn, j))
            nc.sync.dma_start(wn8_sb[:, j, :, :, :], prow(wn8, j))

        def load_mp_streams(m, jp):
            # one ht + one h DMA per (m, block-pair) -- mid-size DMAs keep
            # all 16 DMA engines busy (one DMA maps to ONE engine at
            # ~22GB/s, so whole-tensor transfers serialize); hb1 = h + 1 is
            # precomputed by the otherwise-idle GpSimd right behind the h
            # DMA, well off the epilogue's critical path
            msl = slice(m * P, (m + 1) * P)
            psl = slice(2 * jp * BS, (2 * jp + 2) * BS)
            ht_mp = spool.tile([P, 2 * K2, P], f8, tag="ht_mp")
            nc.sync.dma_start(ht_mp[:, :, :], prow(ht8, jp * mt + m))
            h_mp = spool.tile([P, 2, BS], f16, tag="h_mp")
            nc.sync.dma_start(h_mp[:, :, :], h16[msl, psl])
            hb1_mp = spool.tile([P, 2, BS], f16, tag="hb1_mp")
            nc.gpsimd.tensor_add(hb1_mp[:], h_mp[:], ones2_sb[:])
            return ht_mp, h_mp, hb1_mp

        # prewarm the ACT sigmoid table (~2.7us ACT_TABLE_LOAD) at t~0 so
        # the first real epilogue doesn't pay it inline right when the PE's
        # PSUM bank rotation depends on that sigmoid releasing bank A
        ws = wpool.tile([P, 1], f32, tag="ws")
        nc.vector.memset(ws[:], 0.0)
        nc.scalar.activation(ws[:], ws[:], SIG)
        ones2_sb = wpool.tile([P, 2, BS], f16, tag="ones2_sb")
        nc.gpsimd.memset(ones2_sb[:], 1.0)

        # head, ordered by need-time: pair 0's h-weights + m0 streams, m0's
        # x columns + block 0/1 x-weights, then the remaining per-m streams
        streams = {}
        load_wh_col(0)
        load_wh_col(1)
        streams[(0, 0)] = load_mp_streams(0, 0)
        nc.sync.dma_start(xt8_sb[:, 0, :, :], prow(xt8, 0))
        nc.sync.dma_start(wrz_sb[:, 0, :, :], prow(wrz, 0))
        nc.sync.dma_start(xt16_sb[:, 0, :, :], prow(xt16, 0))
        nc.sync.dma_start(wn_sb[:, 0, :, :], prow(wn, 0))
        nc.sync.dma_start(wn8_sb[:, 0, :, :, :], prow(wn8, 0))
        load_wx_col(1)
        for m in range(1, mt):
            # xt8 + streams feed the early matmul groups; xt16 (wxn path)
            # is needed last within each (j,m), so it loads after them
            nc.sync.dma_start(xt8_sb[:, m, :, :], prow(xt8, m))
            streams[(m, 0)] = load_mp_streams(m, 0)
            nc.sync.dma_start(xt16_sb[:, m, :, :], prow(xt16, m))
        if has_bias:
            ones_sb = wpool.tile([1, P], f32, tag="ones_sb")
            nc.vector.memset(ones_sb[:], 1.0)
            brz_sb = wpool.tile([1, NB * 2 * BS], f32, tag="brz_sb")
            bxn_sb = wpool.tile([1, NB * BS], f32, tag="bxn_sb")
            bhn_sb = wpool.tile([1, NB * BS], f32, tag="bhn_sb")
            nc.sync.dma_start(brz_sb[:], brz_d[:])
            nc.sync.dma_start(bxn_sb[:], bxn_d[:])
            nc.sync.dma_start(bhn_sb[:], bhn_d[:])

        for jp in range(NP):
            for m in range(mt):
                msl = slice(m * P, (m + 1) * P)
                ht_mp, h_mp, hb1_mp = streams.pop((m, jp))
                A2 = psA.tile([P, 2, 2 * BS], f32, tag="A")
                B2 = psB.tile([P, 2, 2 * BS], f32, tag="B")
                for i in range(2):
                    j = 2 * jp + i
                    # h-projection DoubleRow passes open both banks
                    # (start=True marks the bank pending-zero; exactly one
                    # start per bank half)
                    nc.tensor.matmul(
                        A2[:, i, :], lhsT=ht_mp[:, 2 * i : 2 * i + 2, :],
                        rhs=whrz_sb[:, j, :, :],
                        start=True, stop=False, perf_mode=DR,
                    )
                    nc.tensor.matmul(
                        B2[:, i, BS : 2 * BS],
                        lhsT=ht_mp[:, 2 * i : 2 * i + 2, :],
                        rhs=whn_sb[:, j, :, :],
                        start=True, stop=False, perf_mode=DR,
                    )
                    # x-projection r|z: 4 DoubleRow passes (K=256 each)
                    for p in range(K1 // 2):
                        nc.tensor.matmul(
                            A2[:, i, :],
                            lhsT=xt8_sb[:, m, 2 * p : 2 * p + 2, :],
                            rhs=wrz_sb[:, j, 2 * p : 2 * p + 2, :],
                            start=False,
                            stop=(p == K1 // 2 - 1) and not has_bias,
                            perf_mode=DR,
                        )
                    # x-projection n: wxn dominates the error budget, so
                    # CN chunks keep fp16's 10 mantissa bits; the rest run
                    # e4m3 DoubleRow (measured rel err 1.61e-2 at CN=4 vs
                    # 1.18e-2 all-fp16, both under the 2e-2 gate)
                    for k in range(CN):
                        nc.tensor.matmul(
                            B2[:, i, 0:BS], lhsT=xt16_sb[:, m, k, :],
                            rhs=wn_sb[:, j, k, :],
                            start=False, stop=False,
                        )
                    for p in range((K1 - CN) // 2):
                        nc.tensor.matmul(
                            B2[:, i, 0:BS],
                            lhsT=xt8_sb[:, m, CN + 2 * p : CN + 2 * p + 2, :],
                            rhs=wn8_sb[:, j, p, :, :],
                            start=False,
                            stop=(p == (K1 - CN) // 2 - 1) and not has_bias,
                            perf_mode=DR,
                        )
                    if has_bias:
                        jrz = slice(j * 2 * BS, (j + 1) * 2 * BS)
                        jn = slice(j * BS, (j + 1) * BS)
                        # rank-1 bias add: ones[K=1,128].T @ bias[K=1,N]
                        # (biases host-pre-scaled by 4096 to match PSUM units)
                        nc.tensor.matmul(
                            A2[:, i, :], lhsT=ones_sb[:, :], rhs=brz_sb[:, jrz],
                            start=False, stop=True,
                        )
                        nc.tensor.matmul(
                            B2[:, i, 0:BS], lhsT=ones_sb[:, :], rhs=bxn_sb[:, jn],
                            start=False, stop=False,
                        )
                        nc.tensor.matmul(
                            B2[:, i, BS : 2 * BS], lhsT=ones_sb[:, :],
                            rhs=bhn_sb[:, jn],
                            start=False, stop=True,
                        )

                # pair-wide epilogue on ACT + DVE only; fp16 off-PSUM so the
                # back-half DVE ops run the 2x_1port mode. The final few
                # m-tiles emit per-block (half-width) epilogues instead:
                # the drain after the last matmul is bounded by the serial
                # sigma->t3->..->store chain, and halving the op width
                # halves that latency.
                def epilogue(i0, ni, tg):
                    isl = slice(i0, i0 + ni)
                    rz2 = epool.tile([P, ni, 2 * BS], f16, tag="rz" + tg)
                    nc.scalar.activation(
                        rz2[:], A2[:, isl, :], SIG, scale=1.0 / SC
                    )
                    t3 = epool.tile([P, ni, BS], f16, tag="t3" + tg)
                    nc.vector.tensor_mul(
                        t3[:], rz2[:, :, 0:BS], B2[:, isl, BS : 2 * BS]
                    )
                    t4 = epool.tile([P, ni, BS], f16, tag="t4" + tg)
                    nc.vector.tensor_add(t4[:], B2[:, isl, 0:BS], t3[:])
                    tn = epool.tile([P, ni, BS], f16, tag="tn" + tg)
                    nc.scalar.activation(tn[:], t4[:], SIG, scale=2.0 / SC)
                    # n - hb = 2*sigmoid(2y) - (hb + 1), one STT
                    e = epool.tile([P, ni, BS], f16, tag="e" + tg)
                    nc.vector.scalar_tensor_tensor(
                        e[:], tn[:], 2.0, hb1_mp[:, isl, :], op0=MULT, op1=SUB
                    )
                    t5 = epool.tile([P, ni, BS], f16, tag="t5" + tg)
                    nc.vector.tensor_mul(t5[:], rz2[:, :, BS : 2 * BS], e[:])
                    oj = epool.tile([P, ni, BS], f16, tag="t3" + tg)
                    nc.vector.tensor_add(oj[:], t5[:], h_mp[:, isl, :])
                    # stores ride the ACT ring except the final pair (sync
                    # is idle by then, ACT still has a backlog in its FIFO)
                    osl = slice((2 * jp + i0) * BS, (2 * jp + i0 + ni) * BS)
                    if jp == NP - 1:
                        nc.sync.dma_start(out[msl, osl], oj[:])
                    else:
                        nc.scalar.dma_start(out[msl, osl], oj[:])

                epilogue(0, 2, "")
                # this m's pair tiles just released: prefetch its next-pair
                # streams now so the slot-wait never blocks the DMA FIFO
                if jp + 1 < NP:
                    streams[(m, jp + 1)] = load_mp_streams(m, jp + 1)
                # next pair's weight columns, spread over the early m-tiles
                if jp + 1 < NP and m < 2:
                    load_wh_col(2 * (jp + 1) + m)
                    load_wx_col(2 * (jp + 1) + m)

    nc.compile()
    _BUILD_CACHE[key] = nc
    return nc


def _q8(a, scale):
    return np.clip(np.float32(a) * np.float32(scale), -240.0, 240.0).astype(
        ml_dtypes.float8_e4m3
    )


def prep_inputs(x, h, W_ir, b_ir_lin, b_ir, W_h, b_hr, ncores=NCORES):
    """Host-side reshaping/casting -> per-core in_maps + has_bias flag."""
    x = np.asarray(x, dtype=np.float32)
    h = np.asarray(h, dtype=np.float32)
    W_ir = np.asarray(W_ir, dtype=np.float32)
    W_h = np.asarray(W_h, dtype=np.float32)
    b_ir_lin = np.asarray(b_ir_lin, dtype=np.float32)
    b_ir = np.asarray(b_ir, dtype=np.float32)
    b_hr = np.asarray(b_hr, dtype=np.float32)

    bc = x.shape[0] // ncores
    mt = bc // P

    # weights: gate-and-block reordered, pre-scaled, then re-tiled so each
    # block column is one [128, contiguous] DMA panel
    Wr = W_ir[0:H].reshape(NB, BS, IN)
    Wz = W_ir[H : 2 * H].reshape(NB, BS, IN)
    Wn_ = W_ir[2 * H :].reshape(NB, BS, IN)
    wrz_f = (
        np.concatenate([Wr, Wz], axis=1)  # [NB, 512, IN]
        .transpose(2, 0, 1)
        .reshape(IN, NB * 2 * BS)
    )
    wn_f = Wn_.transpose(2, 0, 1).reshape(IN, NB * BS) * SW
    whrz_f = W_h[:, 0 : 2 * BS, :].transpose(2, 0, 1).reshape(BS, NB * 2 * BS)
    whn_f = W_h[:, 2 * BS :, :].transpose(2, 0, 1).reshape(BS, NB * BS)

    def wtile(w, kk, cols):  # [kk*P, NB*cols] -> [NB*P, kk*cols] block-major
        return np.ascontiguousarray(
            w.reshape(kk, P, NB, cols).transpose(2, 1, 0, 3).reshape(NB * P, kk * cols)
        )

    wrz = wtile(_q8(wrz_f, SW), K1, 2 * BS)
    # wxn: first CN k-chunks in fp16 (pre-scaled), remainder in e4m3
    wn = wtile(wn_f[0 : CN * P].astype(np.float16), CN, BS)
    wn8 = wtile(_q8(wn_f[CN * P :] / SW, SW), K1 - CN, BS)
    whrz = wtile(_q8(whrz_f, SW), K2, 2 * BS)
    whn = wtile(_q8(whn_f, SW), K2, BS)

    bx = b_ir_lin + b_ir
    bh = b_hr.reshape(NB, 3 * BS)
    brz = np.concatenate(
        [
            bx[0:H].reshape(NB, BS) + bh[:, 0:BS],
            bx[H : 2 * H].reshape(NB, BS) + bh[:, BS : 2 * BS],
        ],
        axis=1,
    ).reshape(1, NB * 2 * BS)
    bxn = bx[2 * H :].reshape(1, NB * BS).copy()
    bhn = bh[:, 2 * BS :].reshape(1, NB * BS).copy()
    has_bias = bool(np.any(brz) or np.any(bxn) or np.any(bhn))

    xT = np.ascontiguousarray(x.T)  # [IN, B]
    hT = np.ascontiguousarray(h.T)  # [H, B]
    xT8 = _q8(xT, SX)
    xT16 = (xT * SX).astype(np.float16)
    hT8 = _q8(hT, SX)

    def xtile(a, csl):  # [K1*P, bc] -> [mt*P, K1*P] m-major panels
        return np.ascontiguousarray(
            a[:, csl]
            .reshape(K1, P, mt, P)
            .transpose(2, 1, 0, 3)
            .reshape(mt * P, K1 * P)
        )

    def htile(a, csl):  # [NP*4*P, bc] -> [NP*mt*P, 4*P] (pair,m)-major
        return np.ascontiguousarray(
            a[:, csl]
            .reshape(NP, 2 * K2, P, mt, P)
            .transpose(0, 3, 2, 1, 4)
            .reshape(NP * mt * P, 2 * K2 * P)
        )

    in_maps = []
    for c in range(ncores):
        csl = slice(c * bc, (c + 1) * bc)
        m = {
            "xt8": xtile(xT8, csl),
            "xt16": xtile(xT16, csl),
            "ht8": htile(hT8, csl),
            "h16": np.ascontiguousarray(h[csl].astype(np.float16)),
            "wrz": wrz,
            "wn": wn,
            "wn8": wn8,
            "whrz": whrz,
            "whn": whn,
        }
        if has_bias:
            # PSUM carries 4096x the logical value, so biases do too
            m["brz"] = (brz * SC).astype(np.float32)
            m["bxn"] = (bxn * SC).astype(np.float32)
            m["bhn"] = (bhn * SC).astype(np.float32)
        in_maps.append(m)
    return in_maps, has_bias, bc


def kernel(x, h, W_ir, b_ir_lin, b_ir, W_h, b_hr):
    from concourse.bass_utils import run_bass_kernel_spmd

    in_maps, has_bias, bc = prep_inputs(x, h, W_ir, b_ir_lin, b_ir, W_h, b_hr)
    nc = build_nc(bc=bc, has_bias=has_bias)
    try:
        res = run_bass_kernel_spmd(nc, in_maps, list(range(NCORES)))
    except Exception:
        # transient NRT device errors have been observed once in ~10 runs;
        # a single retry reuses the compiled NEFF
        res = run_bass_kernel_spmd(nc, in_maps, list(range(NCORES)))
    return np.concatenate(
        [res.results[c]["out"] for c in range(NCORES)], axis=0
    ).astype(np.float32)


# revision 58
# speedup vs baseline: 1.0483x; 1.0021x over previous
"""Block-diagonal GRU cell for Trainium2, data-parallel over 8 NeuronCores.

Math (per batch row b, block j of 8, block size 256):
    wx  = x @ W_ir.T + b_ir_lin + b_ir          # [B, 6144], gates r|z|n global-chunked
    wh  = hb_j @ W_h[j].T + b_hr_j              # per block, local r|z|n chunks of 256
    r   = sigmoid(wxr + whr)
    z   = sigmoid(wxz + whz)
    n   = tanh(wxn + r * whn)
    h'  = (1-z)*hb + z*n

Device strategy (per core, batch-sharded 1024 rows):
  - Mixed fp8/fp16 matmuls, chosen from a measured per-path error budget
    (L2-relative output error if only that path is e4m3-quantized):
        wxr 1.3e-3 | wxz 9.8e-3 | wxn 1.55e-2 | whr 6.5e-4 | whz 4.9e-3 | whn 4.1e-3
    wxn dominates, so it stays fp16; the other five paths run e4m3 with
    MatmulPerfMode.DoubleRow (two K=128 chunks per pass, 2x PE rate).
    Measured end-to-end rel err ~1.25e-2 vs the 2e-2 gate.
  - Scaling: e4m3 needs the operands lifted out of denormal range, so
    activations carry x16 and weights x256 (PSUM = 4096 * logical). The
    fp16 wxn operands are scaled identically (exact powers of two), so
    both PSUM banks are uniformly 4096-scaled and the descale folds into
    the two activation-scale factors (1/4096 for r|z, 2/4096 for the
    tanh-as-sigmoid trick).
  - Blocks are processed in PAIRS (j, j+1) per m-tile: the PSUM tiles are
    two banks each (A2 = r|z sums for both blocks, B2 = [wxn|whn] for
    both), so every epilogue op is 512-1024 wide instead of 256-512.
    DVE/ACT ops pay ~200ns fixed latency each; doubling the width halves
    that overhead per element. Stores become one contiguous 512-col DMA
    per pair.
  - Epilogue (per pair): rz=sig(A2); t3=r*whn; t4=wxn+t3; tn=sig(2*t4);
    e=2*tn-hb1 (=n-hb, one STT against the GpSimd-precomputed hb1=h+1);
    t5=z*e; out=t5+h. Intermediates are fp16 so the non-PSUM DVE ops hit
    the 2x_1port mode; h and out are fp16 end-to-end (blend error ~2e-4,
    negligible vs the fp8 matmul error; halves that DMA traffic).
  - hb1 = h+1 runs on the otherwise-idle GpSimd right behind each h-tile
    DMA, far off the epilogue's critical path (Pool's V3 ISA only allows
    plain TENSOR_TENSOR, and its sequencer is too slow for the serial
    chain itself).
  - Every DRAM tensor is host-relaid so each DMA reads AND writes >=512B
    contiguous per partition (m-major x columns, block-major weight
    columns, (pair,m)-major h^T tiles): small strided runs were measured
    to cap the sync HWDGE ring at ~180GB/s, starving the pair pipeline.
  - All loads ride the SP HWDGE ring; stores ride the ACT ring (disjoint
    FIFOs, so prefetch-blocked loads never delay epilogue slot releases).
    Except the final pair: by then the sync ring is idle while ACT still
    has work queued ahead in its FIFO.
"""

import sys

if "/opt/trn_rl_repo" not in sys.path:
    sys.path.insert(0, "/opt/trn_rl_repo")

import numpy as np
import ml_dtypes

B, IN, H, NB = 8192, 1024, 2048, 8
BS = H // NB  # 256
NCORES = 8
BC = B // NCORES  # 1024 rows per core
P = 128
K1 = IN // P  # 8 x-projection contraction chunks
K2 = BS // P  # 2 h-projection contraction chunks per block
MT = BC // P  # 8 m-tiles per core
NP = NB // 2  # 4 block-pairs
CN = 4  # wxn k-chunks kept in fp16; the rest run e4m3 DoubleRow
SX = 16.0  # activation pre-scale (fp8 and fp16 operands)
SW = 256.0  # weight pre-scale
SC = SX * SW  # PSUM carries 4096 * logical value

_BUILD_CACHE = {}


def build_nc(bc=BC, has_bias=False):
    """Build the Bass program for one core (SPMD: same program on all 8)."""
    key = (bc, has_bias)
    if key in _BUILD_CACHE:
        return _BUILD_CACHE[key]

    from contextlib import ExitStack

    import concourse.bacc as bacc
    import concourse.mybir as mybir
    import concourse.tile as tile

    f8 = mybir.dt.float8e4
    f16 = mybir.dt.float16
    f32 = mybir.dt.float32
    SIG = mybir.ActivationFunctionType.Sigmoid
    MULT = mybir.AluOpType.mult
    SUB = mybir.AluOpType.subtract
    DR = mybir.MatmulPerfMode.DoubleRow

    mt = bc // P

    # Bacc (not plain Bass): its compile() runs move_matmul_waits_to_ldweights
    # + generate_event_semaphores, which split multi-sem waits down to the
    # 1-wait-per-instruction TRN2 ISA budget.
    nc = bacc.Bacc(target_bir_lowering=False)

    # all dram tensors are pre-tiled on the host: leading index selects a
    # [128, contiguous] panel
    xt8 = nc.dram_tensor("xt8", [mt * P, K1 * P], f8, kind="ExternalInput").ap()
    xt16 = nc.dram_tensor("xt16", [mt * P, K1 * P], f16, kind="ExternalInput").ap()
    ht8 = nc.dram_tensor(
        "ht8", [NP * mt * P, 2 * K2 * P], f8, kind="ExternalInput"
    ).ap()
    h16 = nc.dram_tensor("h16", [bc, H], f16, kind="ExternalInput").ap()
    wrz = nc.dram_tensor("wrz", [NB * P, K1 * 2 * BS], f8, kind="ExternalInput").ap()
    wn = nc.dram_tensor("wn", [NB * P, CN * BS], f16, kind="ExternalInput").ap()
    wn8 = nc.dram_tensor("wn8", [NB * P, (K1 - CN) * BS], f8, kind="ExternalInput").ap()
    whrz = nc.dram_tensor(
        "whrz", [NB * P, K2 * 2 * BS], f8, kind="ExternalInput"
    ).ap()
    whn = nc.dram_tensor("whn", [NB * P, K2 * BS], f8, kind="ExternalInput").ap()
    if has_bias:
        brz_d = nc.dram_tensor("brz", [1, NB * 2 * BS], f32, kind="ExternalInput").ap()
        bxn_d = nc.dram_tensor("bxn", [1, NB * BS], f32, kind="ExternalInput").ap()
        bhn_d = nc.dram_tensor("bhn", [1, NB * BS], f32, kind="ExternalInput").ap()
    out = nc.dram_tensor("out", [bc, H], f16, kind="ExternalOutput").ap()

    def prow(t, i):
        return t[i * P : (i + 1) * P, :]

    # panel-major views: [128, panel-index, contiguous bytes]
    xt8_v = xt8.rearrange("(m p) c -> p m c", p=P)  # [128, mt, K1*P]
    xt16_v = xt16.rearrange("(m p) c -> p m c", p=P)
    ht8_v = ht8.rearrange("(a p) c -> p a c", p=P)  # [128, NP*mt, 2*K2*P]
    h16_v = h16.rearrange("(m p) c -> p m c", p=P)  # [128, mt, H]

    with tile.TileContext(nc) as tc, ExitStack() as ctx:
        wpool = ctx.enter_context(tc.tile_pool(name="wres", bufs=1))
        spool = ctx.enter_context(tc.tile_pool(name="stream", bufs=mt + mt // 2))
        psA = ctx.enter_context(tc.tile_pool(name="psA", bufs=2, space="PSUM"))
        psB = ctx.enter_context(tc.tile_pool(name="psB", bufs=2, space="PSUM"))
        epool = ctx.enter_context(tc.tile_pool(name="epi", bufs=4))

        # ---- resident tiles (m-major / block-major so every DMA panel is
        # contiguous on both sides) ----
        xt8_sb = wpool.tile([P, mt, K1, P], f8, tag="xt8_sb")
        xt16_sb = wpool.tile([P, mt, K1, P], f16, tag="xt16_sb")
        wrz_sb = wpool.tile([P, NB, K1, 2 * BS], f8, tag="wrz_sb")
        wn_sb = wpool.tile([P, NB, CN, BS], f16, tag="wn_sb")
        wn8_sb = wpool.tile([P, NB, (K1 - CN) // 2, 2, BS], f8, tag="wn8_sb")
        whrz_sb = wpool.tile([P, NB, K2, 2 * BS], f8, tag="whrz_sb")
        whn_sb = wpool.tile([P, NB, K2, BS], f8, tag="whn_sb")

        def load_wh_col(j):
            # h-projection weights: block j's h-side passes are the group
            # openers, so these small columns load first
            nc.sync.dma_start(whrz_sb[:, j, :, :], prow(whrz, j))
            nc.sync.dma_start(whn_sb[:, j, :, :], prow(whn, j))

        def load_wx_col(j):
            nc.sync.dma_start(wrz_sb[:, j, :, :], prow(wrz, j))
            nc.sync.dma_start(wn_sb[:, j, :, :], prow(wn, j))
            nc.sync.dma_start(wn8_sb[:, j, :, :, :], prow(wn8, j))

        def load_mp_streams(m, jp):
            # one ht + one h DMA per (m, block-pair) -- mid-size DMAs keep
            # all 16 DMA engines busy (one DMA maps to ONE engine at
            # ~22GB/s, so whole-tensor transfers serialize); hb1 = h + 1 is
            # precomputed by the otherwise-idle GpSimd right behind the h
            # DMA, well off the epilogue's critical path
            msl = slice(m * P, (m + 1) * P)
            psl = slice(2 * jp * BS, (2 * jp + 2) * BS)
            ht_mp = spool.tile([P, 2 * K2, P], f8, tag="ht_mp")
            nc.sync.dma_start(ht_mp[:, :, :], prow(ht8, jp * mt + m))
            h_mp = spool.tile([P, 2, BS], f16, tag="h_mp")
            nc.sync.dma_start(h_mp[:, :, :], h16[msl, psl])
            hb1_mp = spool.tile([P, 2, BS], f16, tag="hb1_mp")
            nc.gpsimd.tensor_add(hb1_mp[:], h_mp[:], ones2_sb[:])
            return ht_mp, h_mp, hb1_mp

        # prewarm the ACT sigmoid table (~2.7us ACT_TABLE_LOAD) at t~0 so
        # the first real epilogue doesn't pay it inline right when the PE's
        # PSUM bank rotation depends on that sigmoid releasing bank A
        ws = wpool.tile([P, 1], f32, tag="ws")
        nc.vector.memset(ws[:], 0.0)
        nc.scalar.activation(ws[:], ws[:], SIG)
        ones2_sb = wpool.tile([P, 2, BS], f16, tag="ones2_sb")
        nc.gpsimd.memset(ones2_sb[:], 1.0)

        # head, ordered by need-time: pair 0's h-weights + m0 streams, m0's
        # x columns + block 0/1 x-weights, then the remaining per-m streams
        streams = {}
        load_wh_col(0)
        load_wh_col(1)
        streams[(0, 0)] = load_mp_streams(0, 0)
        nc.sync.dma_start(xt8_sb[:, 0, :, :], prow(xt8, 0))
        nc.sync.dma_start(wrz_sb[:, 0, :, :], prow(wrz, 0))
        nc.sync.dma_start(xt16_sb[:, 0, :, :], prow(xt16, 0))
        nc.sync.dma_start(wn_sb[:, 0, :, :], prow(wn, 0))
        nc.sync.dma_start(wn8_sb[:, 0, :, :, :], prow(wn8, 0))
        load_wx_col(1)
        for m in range(1, mt):
            # xt8 + streams feed the early matmul groups; xt16 (wxn path)
            # is needed last within each (j,m), so it loads after them
            nc.sync.dma_start(xt8_sb[:, m, :, :], prow(xt8, m))
            streams[(m, 0)] = load_mp_streams(m, 0)
            nc.sync.dma_start(xt16_sb[:, m, :, :], prow(xt16, m))
        if has_bias:
            ones_sb = wpool.tile([1, P], f32, tag="ones_sb")
            nc.vector.memset(ones_sb[:], 1.0)
            brz_sb = wpool.tile([1, NB * 2 * BS], f32, tag="brz_sb")
            bxn_sb = wpool.tile([1, NB * BS], f32, tag="bxn_sb")
            bhn_sb = wpool.tile([1, NB * BS], f32, tag="bhn_sb")
            nc.sync.dma_start(brz_sb[:], brz_d[:])
            nc.sync.dma_start(bxn_sb[:], bxn_d[:])
            nc.sync.dma_start(bhn_sb[:], bhn_d[:])

        for jp in range(NP):
            for m in range(mt):
                msl = slice(m * P, (m + 1) * P)
                ht_mp, h_mp, hb1_mp = streams.pop((m, jp))
                A2 = psA.tile([P, 2, 2 * BS], f32, tag="A")
                B2 = psB.tile([P, 2, 2 * BS], f32, tag="B")
                for i in range(2):
                    j = 2 * jp + i
                    # h-projection DoubleRow passes open both banks
                    # (start=True marks the bank pending-zero; exactly one
                    # start per bank half)
                    nc.tensor.matmul(
                        A2[:, i, :], lhsT=ht_mp[:, 2 * i : 2 * i + 2, :],
                        rhs=whrz_sb[:, j, :, :],
                        start=True, stop=False, perf_mode=DR,
                    )
                    nc.tensor.matmul(
                        B2[:, i, BS : 2 * BS],
                        lhsT=ht_mp[:, 2 * i : 2 * i + 2, :],
                        rhs=whn_sb[:, j, :, :],
                        start=True, stop=False, perf_mode=DR,
                    )
                    # x-projection r|z: 4 DoubleRow passes (K=256 each)
                    for p in range(K1 // 2):
                        nc.tensor.matmul(
                            A2[:, i, :],
                            lhsT=xt8_sb[:, m, 2 * p : 2 * p + 2, :],
                            rhs=wrz_sb[:, j, 2 * p : 2 * p + 2, :],
                            start=False,
                            stop=(p == K1 // 2 - 1) and not has_bias,
                            perf_mode=DR,
                        )
                    # x-projection n: wxn dominates the error budget, so
                    # CN chunks keep fp16's 10 mantissa bits; the rest run
                    # e4m3 DoubleRow (measured rel err 1.61e-2 at CN=4 vs
                    # 1.18e-2 all-fp16, both under the 2e-2 gate)
                    for k in range(CN):
                        nc.tensor.matmul(
                            B2[:, i, 0:BS], lhsT=xt16_sb[:, m, k, :],
                            rhs=wn_sb[:, j, k, :],
                            start=False, stop=False,
                        )
                    for p in range((K1 - CN) // 2):
                        nc.tensor.matmul(
                            B2[:, i, 0:BS],
                            lhsT=xt8_sb[:, m, CN + 2 * p : CN + 2 * p + 2, :],
                            rhs=wn8_sb[:, j, p, :, :],
                            start=False,
                            stop=(p == (K1 - CN) // 2 - 1) and not has_bias,
                            perf_mode=DR,
                        )
                    if has_bias:
                        jrz = slice(j * 2 * BS, (j + 1) * 2 * BS)
                        jn = slice(j * BS, (j + 1) * BS)
                        # rank-1 bias add: ones[K=1,128].T @ bias[K=1,N]
                        # (biases host-pre-scaled by 4096 to match PSUM units)
                        nc.tensor.matmul(
                            A2[:, i, :], lhsT=ones_sb[:, :], rhs=brz_sb[:, jrz],
                            start=False, stop=True,
                        )
                        nc.tensor.matmul(
                            B2[:, i, 0:BS], lhsT=ones_sb[:, :], rhs=bxn_sb[:, jn],
                            start=False, stop=False,
                        )
                        nc.tensor.matmul(
                            B2[:, i, BS : 2 * BS], lhsT=ones_sb[:, :],
                            rhs=bhn_sb[:, jn],
                            start=False, stop=True,
                        )

                # pair-wide epilogue on ACT + DVE only; fp16 off-PSUM so the
                # back-half DVE ops run the 2x_1port mode. The final few
                # m-tiles emit per-block (half-width) epilogues instead:
                # the drain after the last matmul is bounded by the serial
                # sigma->t3->..->store chain, and halving the op width
                # halves that latency.
                def epilogue(i0, ni, tg):
                    isl = slice(i0, i0 + ni)
                    rz2 = epool.tile([P, ni, 2 * BS], f16, tag="rz" + tg)
                    nc.scalar.activation(
                        rz2[:], A2[:, isl, :], SIG, scale=1.0 / SC
                    )
                    t3 = epool.tile([P, ni, BS], f16, tag="t3" + tg)
                    nc.vector.tensor_mul(
                        t3[:], rz2[:, :, 0:BS], B2[:, isl, BS : 2 * BS]
                    )
                    t4 = epool.tile([P, ni, BS], f16, tag="t4" + tg)
                    nc.vector.tensor_add(t4[:], B2[:, isl, 0:BS], t3[:])
                    tn = epool.tile([P, ni, BS], f16, tag="tn" + tg)
                    nc.scalar.activation(tn[:], t4[:], SIG, scale=2.0 / SC)
                    # n - hb = 2*sigmoid(2y) - (hb + 1), one STT
                    e = epool.tile([P, ni, BS], f16, tag="e" + tg)
                    nc.vector.scalar_tensor_tensor(
                        e[:], tn[:], 2.0, hb1_mp[:, isl, :], op0=MULT, op1=SUB
                    )
                    t5 = epool.tile([P, ni, BS], f16, tag="t5" + tg)
                    nc.vector.tensor_mul(t5[:], rz2[:, :, BS : 2 * BS], e[:])
                    oj = epool.tile([P, ni, BS], f16, tag="t3" + tg)
                    nc.vector.tensor_add(oj[:], t5[:], h_mp[:, isl, :])
                    # stores ride the ACT ring except the final pair (sync
                    # is idle by then, ACT still has a backlog in its FIFO)
                    osl = slice((2 * jp + i0) * BS, (2 * jp + i0 + ni) * BS)
                    if jp == NP - 1:
                        nc.sync.dma_start(out[msl, osl], oj[:])
                    else:
                        nc.scalar.dma_start(out[msl, osl], oj[:])

                epilogue(0, 2, "")
                # this m's pair tiles just released: prefetch its next-pair
                # streams now so the slot-wait never blocks the DMA FIFO
                if jp + 1 < NP:
                    streams[(m, jp + 1)] = load_mp_streams(m, jp + 1)
                # next pair's weight columns, spread over the early m-tiles
                if jp + 1 < NP and m < 2:
                    load_wh_col(2 * (jp + 1) + m)
                    load_wx_col(2 * (jp + 1) + m)

    nc.compile()
    _BUILD_CACHE[key] = nc
    return nc


def _q8(a, scale):
    return np.clip(np.float32(a) * np.float32(scale), -240.0, 240.0).astype(
        ml_dtypes.float8_e4m3
    )


def prep_inputs(x, h, W_ir, b_ir_lin, b_ir, W_h, b_hr, ncores=NCORES):
    """Host-side reshaping/casting -> per-core in_maps + has_bias flag."""
    x = np.asarray(x, dtype=np.float32)
    h = np.asarray(h, dtype=np.float32)
    W_ir = np.asarray(W_ir, dtype=np.float32)
    W_h = np.asarray(W_h, dtype=np.float32)
    b_ir_lin = np.asarray(b_ir_lin, dtype=np.float32)
    b_ir = np.asarray(b_ir, dtype=np.float32)
    b_hr = np.asarray(b_hr, dtype=np.float32)

    bc = x.shape[0] // ncores
    mt = bc // P

    # weights: gate-and-block reordered, pre-scaled, then re-tiled so each
    # block column is one [128, contiguous] DMA panel
    Wr = W_ir[0:H].reshape(NB, BS, IN)
    Wz = W_ir[H : 2 * H].reshape(NB, BS, IN)
    Wn_ = W_ir[2 * H :].reshape(NB, BS, IN)
    wrz_f = (
        np.concatenate([Wr, Wz], axis=1)  # [NB, 512, IN]
        .transpose(2, 0, 1)
        .reshape(IN, NB * 2 * BS)
    )
    wn_f = Wn_.transpose(2, 0, 1).reshape(IN, NB * BS) * SW
    whrz_f = W_h[:, 0 : 2 * BS, :].transpose(2, 0, 1).reshape(BS, NB * 2 * BS)
    whn_f = W_h[:, 2 * BS :, :].transpose(2, 0, 1).reshape(BS, NB * BS)

    def wtile(w, kk, cols):  # [kk*P, NB*cols] -> [NB*P, kk*cols] block-major
        return np.ascontiguousarray(
            w.reshape(kk, P, NB, cols).transpose(2, 1, 0, 3).reshape(NB * P, kk * cols)
        )

    wrz = wtile(_q8(wrz_f, SW), K1, 2 * BS)
    # wxn: first CN k-chunks in fp16 (pre-scaled), remainder in e4m3
    wn = wtile(wn_f[0 : CN * P].astype(np.float16), CN, BS)
    wn8 = wtile(_q8(wn_f[CN * P :] / SW, SW), K1 - CN, BS)
    whrz = wtile(_q8(whrz_f, SW), K2, 2 * BS)
    whn = wtile(_q8(whn_f, SW), K2, BS)

    bx = b_ir_lin + b_ir
    bh = b_hr.reshape(NB, 3 * BS)
    brz = np.concatenate(
        [
            bx[0:H].reshape(NB, BS) + bh[:, 0:BS],
            bx[H : 2 * H].reshape(NB, BS) + bh[:, BS : 2 * BS],
        ],
        axis=1,
    ).reshape(1, NB * 2 * BS)
    bxn = bx[2 * H :].reshape(1, NB * BS).copy()
    bhn = bh[:, 2 * BS :].reshape(1, NB * BS).copy()
    has_bias = bool(np.any(brz) or np.any(bxn) or np.any(bhn))

    xT = np.ascontiguousarray(x.T)  # [IN, B]
    hT = np.ascontiguousarray(h.T)  # [H, B]
    xT8 = _q8(xT, SX)
    xT16 = (xT * SX).astype(np.float16)
    hT8 = _q8(hT, SX)

    def xtile(a, csl):  # [K1*P, bc] -> [mt*P, K1*P] m-major panels
        return np.ascontiguousarray(
            a[:, csl]
            .reshape(K1, P, mt, P)
            .transpose(2, 1, 0, 3)
            .reshape(mt * P, K1 * P)
        )

    def htile(a, csl):  # [NP*4*P, bc] -> [NP*mt*P, 4*P] (pair,m)-major
        return np.ascontiguousarray(
            a[:, csl]
            .reshape(NP, 2 * K2, P, mt, P)
            .transpose(0, 3, 2, 1, 4)
            .reshape(NP * mt * P, 2 * K2 * P)
        )

    in_maps = []
    for c in range(ncores):
        csl = slice(c * bc, (c + 1) * bc)
        m = {
            "xt8": xtile(xT8, csl),
            "xt16": xtile(xT16, csl),
            "ht8": htile(hT8, csl),
            "h16": np.ascontiguousarray(h[csl].astype(np.float16)),
            "wrz": wrz,
            "wn": wn,
            "wn8": wn8,
            "whrz": whrz,
            "whn": whn,
        }
        if has_bias:
            # PSUM carries 4096x the logical value, so biases do too
            m["brz"] = (brz * SC).astype(np.float32)
            m["bxn"] = (bxn * SC).astype(np.float32)
            m["bhn"] = (bhn * SC).astype(np.float32)
        in_maps.append(m)
    return in_maps, has_bias, bc


def kernel(x, h, W_ir, b_ir_lin, b_ir, W_h, b_hr):
    from concourse.bass_utils import run_bass_kernel_spmd

    in_maps, has_bias, bc = prep_inputs(x, h, W_ir, b_ir_lin, b_ir, W_h, b_hr)
    nc = build_nc(bc=bc, has_bias=has_bias)
    try:
        res = run_bass_kernel_spmd(nc, in_maps, list(range(NCORES)))
    except Exception:
        # transient NRT device errors have been observed once in ~10 runs;
        # a single retry reuses the compiled NEFF
        res = run_bass_kernel_spmd(nc, in_maps, list(range(NCORES)))
    return np.concatenate(
        [res.results[c]["out"] for c in range(NCORES)], axis=0
    ).astype(np.float32)
